# revision 47
# baseline (speedup 1.0000x reference)
"""Expert-parallel MoE FFN kernel for 8 trn2 NeuronCores.

Problem (per full input):
  x [4, 8, 512, 1024], audio_feat [4, 256, 1024],
  W1/Wa [8, 1024, 4096], b1 [8, 4096], W2 [8, 4096, 1024], b2 [8, 1024]
  out[b,e,n,:] = gelu_tanh(x[b,e,n] @ W1[e] + b1[e] + mean(audio_feat[b]) @ Wa[e]) @ W2[e] + b2[e]

Sharding: expert-parallel — core e owns expert e (weights + x[:, e] slice);
pooled audio replicated. No collectives needed: shard/gather on host.

The audio bias (mean(audio_feat) @ Wa + b1 — 0.001% of the FLOPs) is
computed on host in fp32 and uploaded as a 64KB bias table; this removes
256 LDWEIGHTS-serialized tiny matmuls (~N=4) per core from the PE stream
and 16MB of Wa DMA traffic per core.

Per-core kernel (matmul operands bf16, two dff blocks' GEMM2 in fp8
DoubleRow per G2F8 below; PSUM/accumulator fp32; output streamed bf16):
  - single pass over all 2048 tokens (weights stream exactly once)
  - dff is blocked 8x512; per block:
      GEMM1           h^T tiles [128 dff, 512 tok]; block 0 runs tb-outer
                      (each tb phase needs only 1MB of x, so the DMA-fed
                      start never starves), later blocks c-outer with a
                      token-major inner loop so the gelu (ACT) of a chunk
                      drains its PSUM bank 24 matmuls before reuse
      GEMM2           one [128 tok, 1024 d] PSUM tile (2 banks) per token
                      tile; a single DVE op accumulates it into the SBUF
                      fp32 accumulator (halves DVE instruction pressure)
  - block-0 operands live in dedicated contiguous DRAM params (8KB/
    partition bursts); warm-up matmuls on a memset tile release the HAM
    clock gate with no DMA dependency; final block emits bf16 output
    tiles DMA'd on two queues.
"""
from contextlib import ExitStack

import ml_dtypes
import numpy as np

import concourse.bass as bass
import concourse.tile as tile
from concourse import bacc, mybir
from concourse.bass_utils import run_bass_kernel_spmd

F32 = mybir.dt.float32
BF16 = mybir.dt.bfloat16
FP8 = mybir.dt.float8e4
AF = mybir.ActivationFunctionType
ALU = mybir.AluOpType

B, E, N, D = 4, 8, 512, 1024
DFF = 4 * D
NA = 256
TOK = B * N            # 2048 tokens per expert
KC = D // 128          # 8 d-chunks
NDFB = 8               # dff blocks
DFB = DFF // NDFB      # 512
NCC = DFB // 128       # 4 c-chunks per block
NTB = 4                # token chunks of 512 (== batch b)
NTS = TOK // 128       # 16 token chunks of 128
NC_CORES = 8

# Partial-fp8 GEMM2: the listed (dff-block -> c-pair) slices run as
# fp8e4 DoubleRow matmuls (2x PE rate). Each fp8 slice adds quantization
# noise; 4 of 16 pairs measures rel_err 1.82e-2 vs the 2e-2 gate (the
# computation is bit-deterministic, so the measured margin is real).
# Both pairs of a block go fp8 together: a DoubleRow adjacent to a bf16
# matmul pays a ~190ns mode-switch penalty, while DR->DR chains run at
# full rate, so DR matmuls must be contiguous (whole-block GEMM2).
# Blocks with an fp8 pair run their whole GEMM2 PSUM at W2S x scale so
# the fp8 weights sit in e4m3's normal range (bf16 chunks scale exactly);
# the accumulate folds 1/W2S back out. Blocks 0 and 7 (startup/tail
# special cases) stay bf16.
G2F8 = {3: (0, 1), 5: (0, 1)}
P8LIST = [(b, p) for b in sorted(G2F8) for p in G2F8[b]]
W2S = 32.0

_cache = {}


def _build():
    nc = bacc.Bacc("TRN2", target_bir_lowering=False, debug=False,
                   num_devices=NC_CORES)

    xT_d = nc.declare_dram_parameter("xT", [NTB, 2, 128, KC // 2, N], BF16,
                                     isOutput=False)
    w1_d = nc.declare_dram_parameter("w1", [NDFB, 128, KC, DFB], BF16, isOutput=False)
    w1b0_d = nc.declare_dram_parameter("w1b0", [NCC, 128, KC, 128], BF16,
                                       isOutput=False)
    w2_d = nc.declare_dram_parameter("w2", [NDFB, 128, NCC, D], BF16, isOutput=False)
    w28_d = nc.declare_dram_parameter("w28", [len(P8LIST), 128, 2, D], FP8,
                                      isOutput=False)
    baud_d = nc.declare_dram_parameter("baud", [128, DFF // 128, B], F32, isOutput=False)
    b2b_d = nc.declare_dram_parameter("b2b", [128, D], F32, isOutput=False)
    out_d = nc.declare_dram_parameter("out", [TOK, D], BF16, isOutput=True)

    with tile.TileContext(nc) as tc, ExitStack() as ctx:
        sb = ctx.enter_context(tc.tile_pool(name="sb", bufs=1))
        ps = ctx.enter_context(
            tc.tile_pool(name="ps", bufs=1, space=bass.MemorySpace.PSUM))

        # ---- small persistent tiles -------------------------------------
        # baud[p, cg, b] = (pooled-audio @ Wa + b1)[b, cg*128+p], host-made
        baud_t = sb.tile([128, DFF // 128, B], F32, name="baud_t")
        b2b_t = sb.tile([128, D], F32, name="b2b_t")
        nc.gpsimd.dma_start(out=b2b_t[:], in_=b2b_d.ap())

        # ---- DMA helpers (one contiguous 8KB/partition load per call) ---
        def dma_w(which, d_param, blk, shape):
            t = sb.tile(shape, BF16, name=f"{which}_{blk}", tag=which, bufs=2)
            nc.sync.dma_start(out=t[:], in_=d_param.ap()[blk])
            return t

        # ---- start-up: hand-ordered DMA queue ---------------------------
        # The start is DMA-throughput-bound, so block 0's tiles are split
        # fine-grained and ordered exactly along the consumption order of
        # the PE stream (block 0 runs GEMM1 tb-outer): w1 c0-chunk ->
        # xT tb0 -> audio bias -> rest of w1 -> remaining token blocks.
        w1c_t = []

        def dma_w1c(c):
            t = sb.tile([128, KC, 128], BF16, name=f"w1c{c}_t")
            nc.sync.dma_start(out=t[:], in_=w1b0_d.ap()[c])
            w1c_t.append(t)

        xTh = [[None, None] for _ in range(NTB)]

        def dma_xth(tb, hf):
            t = sb.tile([128, KC // 2, N], BF16, name=f"xT_{tb}_{hf}")
            nc.sync.dma_start(out=t[:], in_=xT_d.ap()[tb][hf])
            xTh[tb][hf] = t

        dma_w1c(0)
        dma_xth(0, 0)
        dma_xth(0, 1)
        dma_w1c(1)
        nc.sync.dma_start(out=baud_t[:], in_=baud_d.ap())
        dma_w1c(2)
        dma_w1c(3)
        for tb in range(1, NTB):
            dma_xth(tb, 0)
            dma_xth(tb, 1)
        w2_t = dma_w("w2", w2_d, 0, [128, NCC, D])

        def xT_ap(tb, kc):
            return xTh[tb][kc // 4][:, kc % 4, :]

        def w1_ap(blk, w1t, kc, c):
            if blk == 0:
                return w1c_t[c][:, kc, :]
            return w1t[:, kc, c * 128:(c + 1) * 128]

        # ---- PE warm-up -------------------------------------------------
        # Cover the DMA-bound first ~6us with throwaway matmuls so the HAM
        # clock gate is released by the time real work arrives; the warm-up
        # tile is memset on-device so the PE busies from t~=0 with no DMA
        # dependency. The dummy activation pulls the one-time ~2.6us gelu
        # table load off the first real GEMM1 chunk's critical path.
        wu_t = sb.tile([128, 4], BF16, name="wu_t")
        nc.gpsimd.memset(wu_t[:], 0.0)
        wu2_t = sb.tile([128, N], BF16, name="wu2_t")
        nc.gpsimd.memset(wu2_t[:], 0.0)
        scr_t = sb.tile([128, 4], F32, name="scr_t")
        nc.scalar.activation(scr_t[:], wu_t[:], AF.Gelu_apprx_tanh,
                             scale=1.0)
        psW = ps.tile([B, N], F32, name="psW", tag="ps2", bufs=2)
        for _ in range(8):
            nc.tensor.matmul(psW[:], wu_t[:], wu2_t[:],
                             start=True, stop=True)

        # ---- main loop --------------------------------------------------
        oacc = [sb.tile([128, D], F32, name=f"oacc_{t}", tag=f"oacc{t}",
                        bufs=1) for t in range(NTS)]
        w1_t = None  # block 0 reads via w1c0_t / w1r_t
        for blk in range(NDFB):
            first_blk = blk == 0
            last_blk = blk == NDFB - 1
            if not first_blk:
                w1_t = dma_w("w1", w1_d, blk, [128, KC, DFB])
                w2_t = dma_w("w2", w2_d, blk, [128, NCC, D])
            f8p = G2F8.get(blk, ())
            scaled = blk in G2F8
            w28_t = {}
            for cp in f8p:
                i8 = P8LIST.index((blk, cp))
                t8 = sb.tile([128, 2, D], FP8, name=f"w28_{blk}_{cp}",
                             tag=f"w28{cp}", bufs=1)
                nc.sync.dma_start(out=t8[:], in_=w28_d.ap()[i8])
                w28_t[cp] = t8

            # GEMM1: h^T tiles [128 dff, 512 tok]. Block 0 runs tb-outer
            # (each tb phase consumes only xT[tb], so the DMA-fed start
            # never starves); later blocks run c-outer, token-major inner
            # so the gelu (ACT) of token-chunk tb drains its PSUM bank 24
            # matmuls before the next c-chunk needs it. Either way the 4
            # live [128, 512] PSUM tiles rotate through tags ps1_0..3.
            hT = [[None] * NTB for _ in range(NCC)]
            hp8 = {cp: [None] * NTB for cp in f8p}
            order = ([(tb, c) for tb in range(NTB) for c in range(NCC)]
                     if first_blk else
                     [(tb, c) for c in range(NCC) for tb in range(NTB)])
            for n_i, (tb, c) in enumerate(order):
                cg = blk * NCC + c
                p1 = ps.tile([128, N], F32, name=f"ps1_{blk}_{c}_{tb}",
                             tag=f"ps1{n_i % NTB}", bufs=1)
                for kc in range(KC):
                    nc.tensor.matmul(
                        p1[:], w1_ap(blk, w1_t, kc, c),
                        xT_ap(tb, kc),
                        start=(kc == 0), stop=(kc == KC - 1))
                if c // 2 in f8p:
                    cp = c // 2
                    if hp8[cp][tb] is None:
                        hp8[cp][tb] = sb.tile(
                            [128, 2, N], FP8, name=f"hp8_{blk}_{cp}_{tb}",
                            tag=f"hp8{cp}b{tb}", bufs=2)
                    h_out = hp8[cp][tb][:, c % 2, :]
                else:
                    h = sb.tile([128, N], BF16, name=f"hT_{blk}_{c}_{tb}",
                                tag=f"hT{c}b{tb}", bufs=2)
                    hT[c][tb] = h
                    h_out = h[:]
                nc.scalar.activation(
                    h_out, p1[:], AF.Gelu_apprx_tanh,
                    bias=baud_t[:, cg, tb:tb + 1], scale=1.0)

            # GEMM2: one [128 tok, 1024 d] PSUM tile (2 banks) per tsg
            for tsg in range(NTS):
                tb, r = tsg // 4, tsg % 4
                tail = last_blk and tsg == NTS - 1
                if not tail:
                    p2 = ps.tile([128, D], F32, name=f"ps2_{blk}_{tsg}",
                                 tag="ps2", bufs=2)
                    halves = [p2[:, 0:512], p2[:, 512:1024]]
                else:
                    # final tile: two 1-bank tiles in GEMM1's (now idle)
                    # banks so the first half drains+stores while the
                    # second half is still on the PE
                    pt = [ps.tile([128, 512], F32, name=f"ps2t_{dh}",
                                  tag=f"ps1{dh}", bufs=1) for dh in range(2)]
                    halves = [pt[0][:], pt[1][:]]
                obf = (sb.tile([128, D], BF16, name=f"obf_{tsg}", tag="obf",
                               bufs=3) if last_blk else None)
                out_q = nc.scalar if tsg % 2 == 0 else nc.gpsimd
                seq = []
                for cp in range(2):
                    if cp in f8p:
                        seq.append(('f8', cp))
                    else:
                        seq.extend(('bf', 2 * cp + cc) for cc in range(2))
                for dh in range(2):
                    for j, (kind, ci) in enumerate(seq):
                        st, sp = j == 0, j == len(seq) - 1
                        if kind == 'f8':
                            nc.tensor.matmul(
                                halves[dh],
                                hp8[ci][tb][:, :, r * 128:(r + 1) * 128],
                                w28_t[ci][:, :, dh * 512:(dh + 1) * 512],
                                start=st, stop=sp,
                                perf_mode=mybir.MatmulPerfMode.DoubleRow)
                        else:
                            nc.tensor.matmul(
                                halves[dh],
                                hT[ci][tb][:, r * 128:(r + 1) * 128],
                                w2_t[:, ci, dh * 512:(dh + 1) * 512],
                                start=st, stop=sp)
                    if tail:
                        # final tile: finish + store per 512-half so the
                        # first half drains while the second is on the PE
                        sl = slice(dh * 512, (dh + 1) * 512)
                        nc.vector.tensor_add(obf[:, sl], oacc[tsg][:, sl],
                                             halves[dh])
                        row0 = tsg * 128
                        out_q.dma_start(
                            out=out_d.ap()[row0:row0 + 128, sl],
                            in_=obf[:, sl])
                if not tail:
                    if first_blk:
                        nc.vector.tensor_add(oacc[tsg][:], p2[:], b2b_t[:])
                    elif last_blk:
                        # final accumulate emits the bf16 output tile
                        nc.vector.tensor_add(obf[:], oacc[tsg][:], p2[:])
                        row0 = tsg * 128
                        out_q.dma_start(
                            out=out_d.ap()[row0:row0 + 128, :],
                            in_=obf[:])
                    elif scaled:
                        # fp8 block ran at W2S x; fold the scale back out.
                        # An all-DR block has only 864ns of PE work per tsg
                        # vs ~1212ns for the DVE accumulate, so the DVE
                        # would pace the PE; route every 3rd accumulate via
                        # ACT (scale-copy, drains the PSUM) + the
                        # otherwise-idle GpSimd engine (SBUF-only add).
                        if tsg % 3 == 2:
                            tmp = sb.tile([128, D], F32,
                                          name=f"tmp8_{blk}_{tsg}",
                                          tag="tmp8", bufs=1)
                            nc.scalar.activation(tmp[:], p2[:], AF.Copy,
                                                 scale=1.0 / W2S)
                            nc.gpsimd.tensor_add(oacc[tsg][:], oacc[tsg][:],
                                                 tmp[:])
                        else:
                            nc.vector.scalar_tensor_tensor(
                                oacc[tsg][:], p2[:], 1.0 / W2S, oacc[tsg][:],
                                ALU.mult, ALU.add)
                    else:
                        nc.vector.tensor_add(oacc[tsg][:], oacc[tsg][:], p2[:])

    nc.compile()
    return nc


def _get_nc():
    if "nc" not in _cache:
        _cache["nc"] = _build()
    return _cache["nc"]


def _prep_in_maps(x, audio_feat, W1, b1, Wa, W2, b2):
    bf = ml_dtypes.bfloat16
    pooled = audio_feat.mean(axis=1)                          # [B, D]
    in_maps = []
    for e in range(E):
        xT = np.ascontiguousarray(
            x[:, e].reshape(TOK, D).astype(bf)
            .reshape(NTB, N, 2, KC // 2, 128).transpose(0, 2, 4, 3, 1))
        w1 = np.ascontiguousarray(
            W1[e].astype(bf).reshape(KC, 128, NDFB, DFB).transpose(2, 1, 0, 3))
        w1b0 = np.ascontiguousarray(
            w1[0].reshape(128, KC, NCC, 128).transpose(2, 0, 1, 3))
        w2r = W2[e].reshape(NDFB, NCC, 128, D)
        w2s = w2r.copy()
        for fblk in G2F8:
            w2s[fblk] *= W2S
        w2 = np.ascontiguousarray(
            w2s.astype(bf).transpose(0, 2, 1, 3))
        w28 = np.ascontiguousarray(np.stack(
            [w2r[fblk, 2 * cp:2 * cp + 2].transpose(1, 0, 2) * W2S
             for (fblk, cp) in P8LIST])).astype(ml_dtypes.float8_e4m3)
        # fp32 host audio bias: baud[p, cg, b] = (pooled @ Wa + b1)[b, cg*128+p]
        ah = pooled @ Wa[e] + b1[e]                           # [B, DFF]
        baud = np.ascontiguousarray(
            ah.T.reshape(DFF // 128, 128, B).transpose(1, 0, 2)).astype(np.float32)
        in_maps.append({
            "xT": xT,
            "w1": w1,
            "w1b0": w1b0,
            "w2": w2,
            "w28": w28,
            "baud": baud,
            "b2b": np.ascontiguousarray(
                np.broadcast_to(b2[e], (128, D))).astype(np.float32),
        })
    return in_maps


def kernel(x, audio_feat, W1, b1, Wa, W2, b2):
    x = np.asarray(x, dtype=np.float32)
    audio_feat = np.asarray(audio_feat, dtype=np.float32)
    W1 = np.asarray(W1, dtype=np.float32)
    b1 = np.asarray(b1, dtype=np.float32)
    Wa = np.asarray(Wa, dtype=np.float32)
    W2 = np.asarray(W2, dtype=np.float32)
    b2 = np.asarray(b2, dtype=np.float32)

    nc = _get_nc()
    in_maps = _prep_in_maps(x, audio_feat, W1, b1, Wa, W2, b2)
    _cache["in_maps"] = in_maps

    # A prior tenant can leave the accelerator in an unrecoverable state
    # that clears after one failed attempt; retry to absorb that.
    last_err = None
    for attempt in range(3):
        try:
            res = run_bass_kernel_spmd(nc, in_maps, list(range(NC_CORES)))
            break
        except Exception as err:  # noqa: BLE001
            last_err = err
            import time
            time.sleep(2.0)
    else:
        raise last_err

    out = np.empty((B, E, N, D), dtype=np.float32)
    for e in range(E):
        out[:, e] = res.results[e]["out"].astype(np.float32).reshape(B, N, D)
    return out


# revision 49
# speedup vs baseline: 1.0016x; 1.0016x over previous
"""Expert-parallel MoE FFN kernel for 8 trn2 NeuronCores.

Problem (per full input):
  x [4, 8, 512, 1024], audio_feat [4, 256, 1024],
  W1/Wa [8, 1024, 4096], b1 [8, 4096], W2 [8, 4096, 1024], b2 [8, 1024]
  out[b,e,n,:] = gelu_tanh(x[b,e,n] @ W1[e] + b1[e] + mean(audio_feat[b]) @ Wa[e]) @ W2[e] + b2[e]

Sharding: expert-parallel — core e owns expert e (weights + x[:, e] slice);
pooled audio replicated. No collectives needed: shard/gather on host.

The audio bias (mean(audio_feat) @ Wa + b1 — 0.001% of the FLOPs) is
computed on host in fp32 and uploaded as a 64KB bias table; this removes
256 LDWEIGHTS-serialized tiny matmuls (~N=4) per core from the PE stream
and 16MB of Wa DMA traffic per core.

Per-core kernel (matmul operands bf16, two dff blocks' GEMM2 in fp8
DoubleRow per G2F8 below; PSUM/accumulator fp32; output streamed bf16):
  - single pass over all 2048 tokens (weights stream exactly once)
  - dff is blocked 8x512; per block:
      GEMM1           h^T tiles [128 dff, 512 tok]; block 0 runs tb-outer
                      (each tb phase needs only 1MB of x, so the DMA-fed
                      start never starves), later blocks c-outer with a
                      token-major inner loop so the gelu (ACT) of a chunk
                      drains its PSUM bank 24 matmuls before reuse
      GEMM2           one [128 tok, 1024 d] PSUM tile (2 banks) per token
                      tile; a single DVE op accumulates it into the SBUF
                      fp32 accumulator (halves DVE instruction pressure)
  - block-0 operands live in dedicated contiguous DRAM params (8KB/
    partition bursts); warm-up matmuls on a memset tile release the HAM
    clock gate with no DMA dependency; final block emits bf16 output
    tiles DMA'd on two queues.
"""
from contextlib import ExitStack

import ml_dtypes
import numpy as np

import concourse.bass as bass
import concourse.tile as tile
from concourse import bacc, mybir
from concourse.bass_utils import run_bass_kernel_spmd

F32 = mybir.dt.float32
BF16 = mybir.dt.bfloat16
FP8 = mybir.dt.float8e4
AF = mybir.ActivationFunctionType
ALU = mybir.AluOpType

B, E, N, D = 4, 8, 512, 1024
DFF = 4 * D
NA = 256
TOK = B * N            # 2048 tokens per expert
KC = D // 128          # 8 d-chunks
NDFB = 8               # dff blocks
DFB = DFF // NDFB      # 512
NCC = DFB // 128       # 4 c-chunks per block
NTB = 4                # token chunks of 512 (== batch b)
NTS = TOK // 128       # 16 token chunks of 128
NC_CORES = 8

# Partial-fp8 GEMM2: the listed (dff-block -> c-pair) slices run as
# fp8e4 DoubleRow matmuls (2x PE rate). Each fp8 slice adds quantization
# noise; 4 of 16 pairs measures rel_err 1.82e-2 vs the 2e-2 gate (the
# computation is bit-deterministic, so the measured margin is real).
# Both pairs of a block go fp8 together: a DoubleRow adjacent to a bf16
# matmul pays a ~190ns mode-switch penalty, while DR->DR chains run at
# full rate, so DR matmuls must be contiguous (whole-block GEMM2).
# Blocks with an fp8 pair run their whole GEMM2 PSUM at W2S x scale so
# the fp8 weights sit in e4m3's normal range (bf16 chunks scale exactly);
# the accumulate folds 1/W2S back out. Blocks 0 and 7 (startup/tail
# special cases) stay bf16.
G2F8 = {3: (0, 1), 5: (0, 1)}
P8LIST = [(b, p) for b in sorted(G2F8) for p in G2F8[b]]
W2S = 32.0

_cache = {}


def _build():
    nc = bacc.Bacc("TRN2", target_bir_lowering=False, debug=False,
                   num_devices=NC_CORES)

    xT_d = nc.declare_dram_parameter("xT", [NTB, 2, 128, KC // 2, N], BF16,
                                     isOutput=False)
    w1_d = nc.declare_dram_parameter("w1", [NDFB, 128, KC, DFB], BF16, isOutput=False)
    w1b0_d = nc.declare_dram_parameter("w1b0", [NCC, 128, KC, 128], BF16,
                                       isOutput=False)
    w2_d = nc.declare_dram_parameter("w2", [NDFB, 128, NCC, D], BF16, isOutput=False)
    w28_d = nc.declare_dram_parameter("w28", [len(P8LIST), 128, 2, D], FP8,
                                      isOutput=False)
    baud_d = nc.declare_dram_parameter("baud", [128, DFF // 128, B], F32, isOutput=False)
    b2b_d = nc.declare_dram_parameter("b2b", [128, D], F32, isOutput=False)
    out_d = nc.declare_dram_parameter("out", [TOK, D], BF16, isOutput=True)

    with tile.TileContext(nc) as tc, ExitStack() as ctx:
        sb = ctx.enter_context(tc.tile_pool(name="sb", bufs=1))
        ps = ctx.enter_context(
            tc.tile_pool(name="ps", bufs=1, space=bass.MemorySpace.PSUM))

        # ---- small persistent tiles -------------------------------------
        # baud[p, cg, b] = (pooled-audio @ Wa + b1)[b, cg*128+p], host-made
        baud_t = sb.tile([128, DFF // 128, B], F32, name="baud_t")
        b2b_t = sb.tile([128, D], F32, name="b2b_t")
        nc.gpsimd.dma_start(out=b2b_t[:], in_=b2b_d.ap())

        # ---- DMA helpers (one contiguous 8KB/partition load per call) ---
        def dma_w(which, d_param, blk, shape):
            t = sb.tile(shape, BF16, name=f"{which}_{blk}", tag=which, bufs=2)
            nc.sync.dma_start(out=t[:], in_=d_param.ap()[blk])
            return t

        # ---- start-up: hand-ordered DMA queue ---------------------------
        # The start is DMA-throughput-bound, so block 0's tiles are split
        # fine-grained and ordered exactly along the consumption order of
        # the PE stream (block 0 runs GEMM1 tb-outer): w1 c0-chunk ->
        # xT tb0 -> audio bias -> rest of w1 -> remaining token blocks.
        w1c_t = []

        def dma_w1c(c):
            t = sb.tile([128, KC, 128], BF16, name=f"w1c{c}_t")
            nc.sync.dma_start(out=t[:], in_=w1b0_d.ap()[c])
            w1c_t.append(t)

        xTh = [[None, None] for _ in range(NTB)]

        def dma_xth(tb, hf):
            t = sb.tile([128, KC // 2, N], BF16, name=f"xT_{tb}_{hf}")
            nc.sync.dma_start(out=t[:], in_=xT_d.ap()[tb][hf])
            xTh[tb][hf] = t

        dma_w1c(0)
        dma_xth(0, 0)
        dma_xth(0, 1)
        dma_w1c(1)
        nc.sync.dma_start(out=baud_t[:], in_=baud_d.ap())
        dma_w1c(2)
        dma_w1c(3)
        for tb in range(1, NTB):
            dma_xth(tb, 0)
            dma_xth(tb, 1)
        w2_t = dma_w("w2", w2_d, 0, [128, NCC, D])

        def xT_ap(tb, kc):
            return xTh[tb][kc // 4][:, kc % 4, :]

        def w1_ap(blk, w1t, kc, c):
            if blk == 0:
                return w1c_t[c][:, kc, :]
            return w1t[:, kc, c * 128:(c + 1) * 128]

        # ---- PE warm-up -------------------------------------------------
        # Cover the DMA-bound first ~6us with throwaway matmuls so the HAM
        # clock gate is released by the time real work arrives; the warm-up
        # tile is memset on-device so the PE busies from t~=0 with no DMA
        # dependency. The dummy activation pulls the one-time ~2.6us gelu
        # table load off the first real GEMM1 chunk's critical path.
        wu_t = sb.tile([128, 4], BF16, name="wu_t")
        nc.gpsimd.memset(wu_t[:], 0.0)
        wu2_t = sb.tile([128, N], BF16, name="wu2_t")
        nc.gpsimd.memset(wu2_t[:], 0.0)
        scr_t = sb.tile([128, 4], F32, name="scr_t")
        nc.scalar.activation(scr_t[:], wu_t[:], AF.Gelu_apprx_tanh,
                             scale=1.0)
        psW = ps.tile([B, N], F32, name="psW", tag="ps2", bufs=2)
        for _ in range(8):
            nc.tensor.matmul(psW[:], wu_t[:], wu2_t[:],
                             start=True, stop=True)

        # ---- main loop --------------------------------------------------
        oacc = [sb.tile([128, D], F32, name=f"oacc_{t}", tag=f"oacc{t}",
                        bufs=1) for t in range(NTS)]
        w1_t = None  # block 0 reads via w1c_t
        pending = None  # deferred GEMM2 of the previous (fp8) block
        for blk in range(NDFB):
            first_blk = blk == 0
            last_blk = blk == NDFB - 1
            if not first_blk:
                w1_t = dma_w("w1", w1_d, blk, [128, KC, DFB])
                w2_t = dma_w("w2", w2_d, blk, [128, NCC, D])
            f8p = G2F8.get(blk, ())
            scaled = blk in G2F8
            w28_t = {}
            for cp in f8p:
                i8 = P8LIST.index((blk, cp))
                t8 = sb.tile([128, 2, D], FP8, name=f"w28_{blk}_{cp}",
                             tag=f"w28{cp}", bufs=1)
                nc.sync.dma_start(out=t8[:], in_=w28_d.ap()[i8])
                w28_t[cp] = t8

            # GEMM1: h^T tiles [128 dff, 512 tok]. Block 0 runs tb-outer
            # (each tb phase consumes only xT[tb], so the DMA-fed start
            # never starves); later blocks run c-outer, token-major inner
            # so the gelu (ACT) of token-chunk tb drains its PSUM bank 24
            # matmuls before the next c-chunk needs it. Either way the 4
            # live [128, 512] PSUM tiles rotate through tags ps1_0..3.
            hT = [[None] * NTB for _ in range(NCC)]
            hp8 = {cp: [None] * NTB for cp in f8p}
            order = ([(tb, c) for tb in range(NTB) for c in range(NCC)]
                     if first_blk else
                     [(tb, c) for c in range(NCC) for tb in range(NTB)])
            for n_i, (tb, c) in enumerate(order):
                cg = blk * NCC + c
                p1 = ps.tile([128, N], F32, name=f"ps1_{blk}_{c}_{tb}",
                             tag=f"ps1{n_i % NTB}", bufs=1)
                for kc in range(KC):
                    nc.tensor.matmul(
                        p1[:], w1_ap(blk, w1_t, kc, c),
                        xT_ap(tb, kc),
                        start=(kc == 0), stop=(kc == KC - 1))
                if c // 2 in f8p:
                    cp = c // 2
                    if hp8[cp][tb] is None:
                        hp8[cp][tb] = sb.tile(
                            [128, 2, N], FP8, name=f"hp8_{blk}_{cp}_{tb}",
                            tag=f"hp8{cp}b{tb}", bufs=2)
                    h_out = hp8[cp][tb][:, c % 2, :]
                else:
                    h = sb.tile([128, N], BF16, name=f"hT_{blk}_{c}_{tb}",
                                tag=f"hT{c}b{tb}", bufs=2)
                    hT[c][tb] = h
                    h_out = h[:]
                nc.scalar.activation(
                    h_out, p1[:], AF.Gelu_apprx_tanh,
                    bias=baud_t[:, cg, tb:tb + 1], scale=1.0)

            # GEMM2: one [128 tok, 1024 d] PSUM tile (2 banks) per tsg.
            # An all-DR (fp8) block has only 864ns of PE work per tsg vs
            # ~1212ns for its DVE accumulate, so emitting its GEMM2 right
            # away would let the DVE pace the PE; instead fp8 blocks defer
            # their GEMM2 until after the NEXT block's GEMM1 (hT/hp8
            # double-buffering is exactly deep enough), spreading the DVE
            # demand over a 41us window. DR matmuls stay contiguous.
            def emit_g2(g_blk, g_hT, g_hp8, g_w28, g_w2t):
                g_f8p = G2F8.get(g_blk, ())
                g_scaled = g_blk in G2F8
                g_first = g_blk == 0
                g_last = g_blk == NDFB - 1
                seq = []
                for cp in range(2):
                    if cp in g_f8p:
                        seq.append(('f8', cp))
                    else:
                        seq.extend(('bf', 2 * cp + cc) for cc in range(2))
                for tsg in range(NTS):
                    tb, r = tsg // 4, tsg % 4
                    tail = g_last and tsg == NTS - 1
                    if not tail:
                        p2 = ps.tile([128, D], F32, name=f"ps2_{g_blk}_{tsg}",
                                     tag="ps2", bufs=2)
                        halves = [p2[:, 0:512], p2[:, 512:1024]]
                    else:
                        # final tile: two 1-bank tiles in GEMM1's (now
                        # idle) banks so the first half drains+stores
                        # while the second half is still on the PE
                        pt = [ps.tile([128, 512], F32, name=f"ps2t_{dh}",
                                      tag=f"ps1{dh}", bufs=1)
                              for dh in range(2)]
                        halves = [pt[0][:], pt[1][:]]
                    obf = (sb.tile([128, D], BF16, name=f"obf_{tsg}",
                                   tag="obf", bufs=3) if g_last else None)
                    out_q = nc.scalar if tsg % 2 == 0 else nc.gpsimd
                    for dh in range(2):
                        for j, (kind, ci) in enumerate(seq):
                            st, sp = j == 0, j == len(seq) - 1
                            if kind == 'f8':
                                nc.tensor.matmul(
                                    halves[dh],
                                    g_hp8[ci][tb][:, :, r * 128:(r + 1) * 128],
                                    g_w28[ci][:, :, dh * 512:(dh + 1) * 512],
                                    start=st, stop=sp,
                                    perf_mode=mybir.MatmulPerfMode.DoubleRow)
                            else:
                                nc.tensor.matmul(
                                    halves[dh],
                                    g_hT[ci][tb][:, r * 128:(r + 1) * 128],
                                    g_w2t[:, ci, dh * 512:(dh + 1) * 512],
                                    start=st, stop=sp)
                        if tail:
                            # finish + store per 512-half so the first
                            # half drains while the second is on the PE
                            sl = slice(dh * 512, (dh + 1) * 512)
                            nc.vector.tensor_add(obf[:, sl],
                                                 oacc[tsg][:, sl],
                                                 halves[dh])
                            row0 = tsg * 128
                            out_q.dma_start(
                                out=out_d.ap()[row0:row0 + 128, sl],
                                in_=obf[:, sl])
                    if not tail:
                        if g_first:
                            nc.vector.tensor_add(oacc[tsg][:], p2[:],
                                                 b2b_t[:])
                        elif g_last:
                            # final accumulate emits the bf16 output tile
                            nc.vector.tensor_add(obf[:], oacc[tsg][:], p2[:])
                            row0 = tsg * 128
                            out_q.dma_start(
                                out=out_d.ap()[row0:row0 + 128, :],
                                in_=obf[:])
                        elif g_scaled:
                            # fp8 block ran at W2S x; fold the scale out
                            nc.vector.scalar_tensor_tensor(
                                oacc[tsg][:], p2[:], 1.0 / W2S,
                                oacc[tsg][:], ALU.mult, ALU.add)
                        else:
                            nc.vector.tensor_add(oacc[tsg][:], oacc[tsg][:],
                                                 p2[:])

            if pending is not None:
                emit_g2(*pending)
                pending = None
            if scaled:
                pending = (blk, hT, hp8, w28_t, w2_t)
            else:
                emit_g2(blk, hT, hp8, w28_t, w2_t)
        assert pending is None

    nc.compile()
    return nc


def _get_nc():
    if "nc" not in _cache:
        _cache["nc"] = _build()
    return _cache["nc"]


def _prep_in_maps(x, audio_feat, W1, b1, Wa, W2, b2):
    bf = ml_dtypes.bfloat16
    pooled = audio_feat.mean(axis=1)                          # [B, D]
    in_maps = []
    for e in range(E):
        xT = np.ascontiguousarray(
            x[:, e].reshape(TOK, D).astype(bf)
            .reshape(NTB, N, 2, KC // 2, 128).transpose(0, 2, 4, 3, 1))
        w1 = np.ascontiguousarray(
            W1[e].astype(bf).reshape(KC, 128, NDFB, DFB).transpose(2, 1, 0, 3))
        w1b0 = np.ascontiguousarray(
            w1[0].reshape(128, KC, NCC, 128).transpose(2, 0, 1, 3))
        w2r = W2[e].reshape(NDFB, NCC, 128, D)
        w2s = w2r.copy()
        for fblk in G2F8:
            w2s[fblk] *= W2S
        w2 = np.ascontiguousarray(
            w2s.astype(bf).transpose(0, 2, 1, 3))
        w28 = np.ascontiguousarray(np.stack(
            [w2r[fblk, 2 * cp:2 * cp + 2].transpose(1, 0, 2) * W2S
             for (fblk, cp) in P8LIST])).astype(ml_dtypes.float8_e4m3)
        # fp32 host audio bias: baud[p, cg, b] = (pooled @ Wa + b1)[b, cg*128+p]
        ah = pooled @ Wa[e] + b1[e]                           # [B, DFF]
        baud = np.ascontiguousarray(
            ah.T.reshape(DFF // 128, 128, B).transpose(1, 0, 2)).astype(np.float32)
        in_maps.append({
            "xT": xT,
            "w1": w1,
            "w1b0": w1b0,
            "w2": w2,
            "w28": w28,
            "baud": baud,
            "b2b": np.ascontiguousarray(
                np.broadcast_to(b2[e], (128, D))).astype(np.float32),
        })
    return in_maps


def kernel(x, audio_feat, W1, b1, Wa, W2, b2):
    x = np.asarray(x, dtype=np.float32)
    audio_feat = np.asarray(audio_feat, dtype=np.float32)
    W1 = np.asarray(W1, dtype=np.float32)
    b1 = np.asarray(b1, dtype=np.float32)
    Wa = np.asarray(Wa, dtype=np.float32)
    W2 = np.asarray(W2, dtype=np.float32)
    b2 = np.asarray(b2, dtype=np.float32)

    nc = _get_nc()
    in_maps = _prep_in_maps(x, audio_feat, W1, b1, Wa, W2, b2)
    _cache["in_maps"] = in_maps

    # A prior tenant can leave the accelerator in an unrecoverable state
    # that clears after one failed attempt; retry to absorb that.
    last_err = None
    for attempt in range(3):
        try:
            res = run_bass_kernel_spmd(nc, in_maps, list(range(NC_CORES)))
            break
        except Exception as err:  # noqa: BLE001
            last_err = err
            import time
            time.sleep(2.0)
    else:
        raise last_err

    out = np.empty((B, E, N, D), dtype=np.float32)
    for e in range(E):
        out[:, e] = res.results[e]["out"].astype(np.float32).reshape(B, N, D)
    return out


# revision 50
# speedup vs baseline: 1.0089x; 1.0073x over previous
"""Expert-parallel MoE FFN kernel for 8 trn2 NeuronCores.

Problem (per full input):
  x [4, 8, 512, 1024], audio_feat [4, 256, 1024],
  W1/Wa [8, 1024, 4096], b1 [8, 4096], W2 [8, 4096, 1024], b2 [8, 1024]
  out[b,e,n,:] = gelu_tanh(x[b,e,n] @ W1[e] + b1[e] + mean(audio_feat[b]) @ Wa[e]) @ W2[e] + b2[e]

Sharding: expert-parallel — core e owns expert e (weights + x[:, e] slice);
pooled audio replicated. No collectives needed: shard/gather on host.

The audio bias (mean(audio_feat) @ Wa + b1 — 0.001% of the FLOPs) is
computed on host in fp32 and uploaded as a 64KB bias table; this removes
256 LDWEIGHTS-serialized tiny matmuls (~N=4) per core from the PE stream
and 16MB of Wa DMA traffic per core.

Per-core kernel (matmul operands bf16, two dff blocks' GEMM2 in fp8
DoubleRow per G2F8 below; PSUM/accumulator fp32; output streamed bf16):
  - single pass over all 2048 tokens (weights stream exactly once)
  - dff is blocked 8x512; per block:
      GEMM1           h^T tiles [128 dff, 512 tok]; block 0 runs tb-outer
                      (each tb phase needs only 1MB of x, so the DMA-fed
                      start never starves), later blocks c-outer with a
                      token-major inner loop so the gelu (ACT) of a chunk
                      drains its PSUM bank 24 matmuls before reuse
      GEMM2           one [128 tok, 1024 d] PSUM tile (2 banks) per token
                      tile; a single DVE op accumulates it into the SBUF
                      fp32 accumulator (halves DVE instruction pressure)
  - block-0 operands live in dedicated contiguous DRAM params (8KB/
    partition bursts); warm-up matmuls on a memset tile release the HAM
    clock gate with no DMA dependency; final block emits bf16 output
    tiles DMA'd on two queues.
"""
from contextlib import ExitStack

import ml_dtypes
import numpy as np

import concourse.bass as bass
import concourse.tile as tile
from concourse import bacc, mybir
from concourse.bass_utils import run_bass_kernel_spmd

F32 = mybir.dt.float32
BF16 = mybir.dt.bfloat16
FP8 = mybir.dt.float8e4
AF = mybir.ActivationFunctionType
ALU = mybir.AluOpType

B, E, N, D = 4, 8, 512, 1024
DFF = 4 * D
NA = 256
TOK = B * N            # 2048 tokens per expert
KC = D // 128          # 8 d-chunks
NDFB = 8               # dff blocks
DFB = DFF // NDFB      # 512
NCC = DFB // 128       # 4 c-chunks per block
NTB = 4                # token chunks of 512 (== batch b)
NTS = TOK // 128       # 16 token chunks of 128
NC_CORES = 8

# Partial-fp8 GEMM2: the listed (dff-block -> c-pair) slices run as
# fp8e4 DoubleRow matmuls (2x PE rate). Each fp8 slice adds quantization
# noise; 4 of 16 pairs measures rel_err 1.82e-2 vs the 2e-2 gate (the
# computation is bit-deterministic, so the measured margin is real).
# Both pairs of a block go fp8 together: a DoubleRow adjacent to a bf16
# matmul pays a ~190ns mode-switch penalty, while DR->DR chains run at
# full rate, so DR matmuls must be contiguous (whole-block GEMM2).
# Blocks with an fp8 pair run their whole GEMM2 PSUM at W2S x scale so
# the fp8 weights sit in e4m3's normal range (bf16 chunks scale exactly);
# the accumulate folds 1/W2S back out. Blocks 0 and 7 (startup/tail
# special cases) stay bf16.
G2F8 = {3: (0, 1), 5: (0, 1)}
P8LIST = [(b, p) for b in sorted(G2F8) for p in G2F8[b]]
W2S = 32.0

_cache = {}


def _build():
    nc = bacc.Bacc("TRN2", target_bir_lowering=False, debug=False,
                   num_devices=NC_CORES)

    xT_d = nc.declare_dram_parameter("xT", [NTB, 2, 128, KC // 2, N], BF16,
                                     isOutput=False)
    w1_d = nc.declare_dram_parameter("w1", [NDFB, 128, KC, DFB], BF16, isOutput=False)
    w1b0_d = nc.declare_dram_parameter("w1b0", [NCC, 128, KC, 128], BF16,
                                       isOutput=False)
    w2_d = nc.declare_dram_parameter("w2", [NDFB, 128, NCC, D], BF16, isOutput=False)
    w28_d = nc.declare_dram_parameter("w28", [len(P8LIST), 128, 2, D], FP8,
                                      isOutput=False)
    baud_d = nc.declare_dram_parameter("baud", [128, DFF // 128, B], F32, isOutput=False)
    b2b_d = nc.declare_dram_parameter("b2b", [128, D], F32, isOutput=False)
    out_d = nc.declare_dram_parameter("out", [TOK, D], BF16, isOutput=True)

    with tile.TileContext(nc) as tc, ExitStack() as ctx:
        sb = ctx.enter_context(tc.tile_pool(name="sb", bufs=1))
        ps = ctx.enter_context(
            tc.tile_pool(name="ps", bufs=1, space=bass.MemorySpace.PSUM))

        # ---- small persistent tiles -------------------------------------
        # baud[p, cg, b] = (pooled-audio @ Wa + b1)[b, cg*128+p], host-made
        baud_t = sb.tile([128, DFF // 128, B], F32, name="baud_t")
        b2b_t = sb.tile([128, D], F32, name="b2b_t")
        nc.gpsimd.dma_start(out=b2b_t[:], in_=b2b_d.ap())

        # ---- DMA helpers (one contiguous 8KB/partition load per call) ---
        def dma_w(which, d_param, blk, shape):
            t = sb.tile(shape, BF16, name=f"{which}_{blk}", tag=which, bufs=2)
            nc.sync.dma_start(out=t[:], in_=d_param.ap()[blk])
            return t

        # ---- start-up: hand-ordered DMA queue ---------------------------
        # The start is DMA-throughput-bound, so block 0's tiles are split
        # fine-grained and ordered exactly along the consumption order of
        # the PE stream (block 0 runs GEMM1 tb-outer): w1 c0-chunk ->
        # xT tb0 -> audio bias -> rest of w1 -> remaining token blocks.
        w1c_t = []

        def dma_w1c(c):
            t = sb.tile([128, KC, 128], BF16, name=f"w1c{c}_t")
            nc.sync.dma_start(out=t[:], in_=w1b0_d.ap()[c])
            w1c_t.append(t)

        xTh = [[None, None] for _ in range(NTB)]

        def dma_xth(tb, hf):
            t = sb.tile([128, KC // 2, N], BF16, name=f"xT_{tb}_{hf}")
            nc.sync.dma_start(out=t[:], in_=xT_d.ap()[tb][hf])
            xTh[tb][hf] = t

        dma_w1c(0)
        dma_xth(0, 0)
        dma_xth(0, 1)
        dma_w1c(1)
        nc.sync.dma_start(out=baud_t[:], in_=baud_d.ap())
        dma_w1c(2)
        dma_w1c(3)
        for tb in range(1, NTB):
            dma_xth(tb, 0)
            dma_xth(tb, 1)
        w2_t = dma_w("w2", w2_d, 0, [128, NCC, D])

        def xT_ap(tb, kc):
            return xTh[tb][kc // 4][:, kc % 4, :]

        def w1_ap(blk, w1t, kc, c):
            if blk == 0:
                return w1c_t[c][:, kc, :]
            return w1t[:, kc, c * 128:(c + 1) * 128]

        # ---- PE warm-up -------------------------------------------------
        # Cover the DMA-bound first ~6us with throwaway matmuls so the HAM
        # clock gate is released by the time real work arrives; the warm-up
        # tile is memset on-device so the PE busies from t~=0 with no DMA
        # dependency. The dummy activation pulls the one-time ~2.6us gelu
        # table load off the first real GEMM1 chunk's critical path.
        wu_t = sb.tile([128, 4], BF16, name="wu_t")
        nc.gpsimd.memset(wu_t[:], 0.0)
        wu2_t = sb.tile([128, N], BF16, name="wu2_t")
        nc.gpsimd.memset(wu2_t[:], 0.0)
        scr_t = sb.tile([128, 4], F32, name="scr_t")
        nc.scalar.activation(scr_t[:], wu_t[:], AF.Gelu_apprx_tanh,
                             scale=1.0)
        psW = ps.tile([B, N], F32, name="psW", tag="ps2", bufs=2)
        for _ in range(8):
            nc.tensor.matmul(psW[:], wu_t[:], wu2_t[:],
                             start=True, stop=True)

        # ---- main loop --------------------------------------------------
        oacc = [sb.tile([128, D], F32, name=f"oacc_{t}", tag=f"oacc{t}",
                        bufs=1) for t in range(NTS)]
        w1_t = None  # block 0 reads via w1c_t
        pending = None  # deferred GEMM2 of the previous (fp8) block
        for blk in range(NDFB):
            first_blk = blk == 0
            last_blk = blk == NDFB - 1
            if not first_blk:
                w1_t = dma_w("w1", w1_d, blk, [128, KC, DFB])
                w2_t = dma_w("w2", w2_d, blk, [128, NCC, D])
            f8p = G2F8.get(blk, ())
            scaled = blk in G2F8
            w28_t = {}
            for cp in f8p:
                i8 = P8LIST.index((blk, cp))
                t8 = sb.tile([128, 2, D], FP8, name=f"w28_{blk}_{cp}",
                             tag=f"w28{cp}", bufs=1)
                nc.sync.dma_start(out=t8[:], in_=w28_d.ap()[i8])
                w28_t[cp] = t8

            # GEMM1: h^T tiles [128 dff, 512 tok]. Block 0 runs tb-outer
            # (each tb phase consumes only xT[tb], so the DMA-fed start
            # never starves); later blocks run c-outer, token-major inner
            # so the gelu (ACT) of token-chunk tb drains its PSUM bank 24
            # matmuls before the next c-chunk needs it. Either way the 4
            # live [128, 512] PSUM tiles rotate through tags ps1_0..3.
            hT = [[None] * NTB for _ in range(NCC)]
            hp8 = {cp: [None] * NTB for cp in f8p}
            order = ([(tb, c) for tb in range(NTB) for c in range(NCC)]
                     if first_blk else
                     [(tb, c) for c in range(NCC) for tb in range(NTB)])
            for n_i, (tb, c) in enumerate(order):
                cg = blk * NCC + c
                p1 = ps.tile([128, N], F32, name=f"ps1_{blk}_{c}_{tb}",
                             tag=f"ps1{n_i % NTB}", bufs=1)
                for kc in range(KC):
                    nc.tensor.matmul(
                        p1[:], w1_ap(blk, w1_t, kc, c),
                        xT_ap(tb, kc),
                        start=(kc == 0), stop=(kc == KC - 1))
                if c // 2 in f8p:
                    cp = c // 2
                    if hp8[cp][tb] is None:
                        hp8[cp][tb] = sb.tile(
                            [128, 2, N], FP8, name=f"hp8_{blk}_{cp}_{tb}",
                            tag=f"hp8{cp}b{tb}", bufs=2)
                    h_out = hp8[cp][tb][:, c % 2, :]
                else:
                    h = sb.tile([128, N], BF16, name=f"hT_{blk}_{c}_{tb}",
                                tag=f"hT{c}b{tb}", bufs=2)
                    hT[c][tb] = h
                    h_out = h[:]
                nc.scalar.activation(
                    h_out, p1[:], AF.Gelu_apprx_tanh,
                    bias=baud_t[:, cg, tb:tb + 1], scale=1.0)

            # GEMM2: one [128 tok, 1024 d] PSUM tile (2 banks) per tsg.
            # An all-DR (fp8) block has only 864ns of PE work per tsg vs
            # ~1212ns for its DVE accumulate, so emitting its GEMM2 right
            # away would let the DVE pace the PE; instead fp8 blocks defer
            # their GEMM2 until after the NEXT block's GEMM1 (hT/hp8
            # double-buffering is exactly deep enough), spreading the DVE
            # demand over a 41us window. DR matmuls stay contiguous.
            def emit_g2(g_blk, g_hT, g_hp8, g_w28, g_w2t):
                g_f8p = G2F8.get(g_blk, ())
                g_scaled = g_blk in G2F8
                g_first = g_blk == 0
                g_last = g_blk == NDFB - 1
                seq = []
                for cp in range(2):
                    if cp in g_f8p:
                        seq.append(('f8', cp))
                    else:
                        seq.extend(('bf', 2 * cp + cc) for cc in range(2))
                for tsg in range(NTS):
                    tb, r = tsg // 4, tsg % 4
                    tail = g_last and tsg == NTS - 1
                    if not tail:
                        p2 = ps.tile([128, D], F32, name=f"ps2_{g_blk}_{tsg}",
                                     tag="ps2", bufs=2)
                        halves = [p2[:, 0:512], p2[:, 512:1024]]
                    else:
                        # final tile: two 1-bank tiles in GEMM1's (now
                        # idle) banks so the first half drains+stores
                        # while the second half is still on the PE
                        pt = [ps.tile([128, 512], F32, name=f"ps2t_{dh}",
                                      tag=f"ps1{dh}", bufs=1)
                              for dh in range(2)]
                        halves = [pt[0][:], pt[1][:]]
                    obf = (sb.tile([128, D], BF16, name=f"obf_{tsg}",
                                   tag="obf", bufs=3) if g_last else None)
                    out_q = nc.scalar if tsg % 2 == 0 else nc.gpsimd
                    for dh in range(2):
                        for j, (kind, ci) in enumerate(seq):
                            st, sp = j == 0, j == len(seq) - 1
                            if kind == 'f8':
                                nc.tensor.matmul(
                                    halves[dh],
                                    g_hp8[ci][tb][:, :, r * 128:(r + 1) * 128],
                                    g_w28[ci][:, :, dh * 512:(dh + 1) * 512],
                                    start=st, stop=sp,
                                    perf_mode=mybir.MatmulPerfMode.DoubleRow)
                            else:
                                nc.tensor.matmul(
                                    halves[dh],
                                    g_hT[ci][tb][:, r * 128:(r + 1) * 128],
                                    g_w2t[:, ci, dh * 512:(dh + 1) * 512],
                                    start=st, stop=sp)
                        if tail:
                            # finish + store per 512-half so the first
                            # half drains while the second is on the PE
                            sl = slice(dh * 512, (dh + 1) * 512)
                            nc.vector.tensor_add(obf[:, sl],
                                                 oacc[tsg][:, sl],
                                                 halves[dh])
                            row0 = tsg * 128
                            out_q.dma_start(
                                out=out_d.ap()[row0:row0 + 128, sl],
                                in_=obf[:, sl])
                    if not tail:
                        if g_first:
                            nc.vector.tensor_add(oacc[tsg][:], p2[:],
                                                 b2b_t[:])
                        elif g_last:
                            # final accumulate emits the bf16 output tile
                            nc.vector.tensor_add(obf[:], oacc[tsg][:], p2[:])
                            row0 = tsg * 128
                            out_q.dma_start(
                                out=out_d.ap()[row0:row0 + 128, :],
                                in_=obf[:])
                        elif g_scaled:
                            # fp8 block ran at W2S x; fold the scale out
                            nc.vector.scalar_tensor_tensor(
                                oacc[tsg][:], p2[:], 1.0 / W2S,
                                oacc[tsg][:], ALU.mult, ALU.add)
                        else:
                            nc.vector.tensor_add(oacc[tsg][:], oacc[tsg][:],
                                                 p2[:])

            # (Deferring an fp8 block's GEMM2 past the next block's GEMM1
            # to relieve DVE pacing measured ~3us SLOWER; emit in order.)
            emit_g2(blk, hT, hp8, w28_t, w2_t)
        assert pending is None

    nc.compile()
    return nc


def _get_nc():
    if "nc" not in _cache:
        _cache["nc"] = _build()
    return _cache["nc"]


def _prep_in_maps(x, audio_feat, W1, b1, Wa, W2, b2):
    bf = ml_dtypes.bfloat16
    pooled = audio_feat.mean(axis=1)                          # [B, D]
    in_maps = []
    for e in range(E):
        xT = np.ascontiguousarray(
            x[:, e].reshape(TOK, D).astype(bf)
            .reshape(NTB, N, 2, KC // 2, 128).transpose(0, 2, 4, 3, 1))
        w1 = np.ascontiguousarray(
            W1[e].astype(bf).reshape(KC, 128, NDFB, DFB).transpose(2, 1, 0, 3))
        w1b0 = np.ascontiguousarray(
            w1[0].reshape(128, KC, NCC, 128).transpose(2, 0, 1, 3))
        w2r = W2[e].reshape(NDFB, NCC, 128, D)
        w2s = w2r.copy()
        for fblk in G2F8:
            w2s[fblk] *= W2S
        w2 = np.ascontiguousarray(
            w2s.astype(bf).transpose(0, 2, 1, 3))
        w28 = np.ascontiguousarray(np.stack(
            [w2r[fblk, 2 * cp:2 * cp + 2].transpose(1, 0, 2) * W2S
             for (fblk, cp) in P8LIST])).astype(ml_dtypes.float8_e4m3)
        # fp32 host audio bias: baud[p, cg, b] = (pooled @ Wa + b1)[b, cg*128+p]
        ah = pooled @ Wa[e] + b1[e]                           # [B, DFF]
        baud = np.ascontiguousarray(
            ah.T.reshape(DFF // 128, 128, B).transpose(1, 0, 2)).astype(np.float32)
        in_maps.append({
            "xT": xT,
            "w1": w1,
            "w1b0": w1b0,
            "w2": w2,
            "w28": w28,
            "baud": baud,
            "b2b": np.ascontiguousarray(
                np.broadcast_to(b2[e], (128, D))).astype(np.float32),
        })
    return in_maps


def kernel(x, audio_feat, W1, b1, Wa, W2, b2):
    x = np.asarray(x, dtype=np.float32)
    audio_feat = np.asarray(audio_feat, dtype=np.float32)
    W1 = np.asarray(W1, dtype=np.float32)
    b1 = np.asarray(b1, dtype=np.float32)
    Wa = np.asarray(Wa, dtype=np.float32)
    W2 = np.asarray(W2, dtype=np.float32)
    b2 = np.asarray(b2, dtype=np.float32)

    nc = _get_nc()
    in_maps = _prep_in_maps(x, audio_feat, W1, b1, Wa, W2, b2)
    _cache["in_maps"] = in_maps

    # A prior tenant can leave the accelerator in an unrecoverable state
    # that clears after one failed attempt; retry to absorb that.
    last_err = None
    for attempt in range(3):
        try:
            res = run_bass_kernel_spmd(nc, in_maps, list(range(NC_CORES)))
            break
        except Exception as err:  # noqa: BLE001
            last_err = err
            import time
            time.sleep(2.0)
    else:
        raise last_err

    out = np.empty((B, E, N, D), dtype=np.float32)
    for e in range(E):
        out[:, e] = res.results[e]["out"].astype(np.float32).reshape(B, N, D)
    return out
